# revision 24
# baseline (speedup 1.0000x reference)
"""Trainium2 Bass kernel for nn_IterativeClassifier (B=65536, D=512, E=64, C=10, T=40).

Pure data parallel over 8 cores (batch-sharded). Per core, activations live
TRANSPOSED on-chip: [E, batch] with batch on the free dim; two batch tiles of
512 are paired onto 128 partitions (tile A rows 0:64, B rows 64:128), and two
pairs share one [128,1024] 2-bank PSUM tile (a "quad", 4 quads = all 8 banks).

Using relu positive-homogeneity and the de-scaled substitution
h^_t := 0.9^-t h_t, the whole 40-step loop runs on ONE persistent PSUM
accumulator P per pair that is never restarted:
    P_0 = u + W1z@z0          u := (W1f@W_feat) @ x   (fused on host)
    P_t = P_{t-1} + G'@h^_{t-1} + s_t*u     G' = (0.1/0.9) W1z@W2,
                                            s_t = 0.1*0.9^-t
    h^_t = relu(P_t + beta_t)               (evac, PSUM -> fp16 SBUF)
All matmuls take fp16 inputs (1 cy/row on the PE, vs 4 for fp32) with
128-wide block-diagonal stationaries, so a step is exactly TWO N=512 matmul
instructions per pair = 16 per step per core (~3.44us, PE-dense).

The logits matmul stays OFF the PE during the loop: S := sum_t h^_t is
accumulated in fp16 and at the end
    logits = 0.9^39 * (BD(0.1*CE@W2)@S + BD(0.9*CE)@z0) + biasL.
Per-step engine schedule (all under the PE's 3.44us):
  - evacs: Vector takes quad 0 (first in PE order), Scalar quads 1-3;
  - S-accumulate: quads 0,1 as Vector tensor_tensor (interleaved between
    the Vector evac and the Scalar evacs' consumers), quads 2,3 on the
    gpsimd SW-DGE DMA queue with accum_op=add, double-buffered into
    even/odd accumulators so the per-tile DMA WAW chain (~2.4us
    latency/hop) spans two steps and never backs up.
x is shipped as fp16 [128, 2048] tiles (8 MB/core, DMA-bound feature phase)
with 8 tiles of prefetch depth; hh tiles are 8-deep per quad so evac
slot-reuse never waits on the (lagging) S-DMA queue.
"""

import numpy as np

import concourse.bass as bass
import concourse.bacc as bacc
import concourse.mybir as mybir
import concourse.tile as tile
from concourse.bass_utils import run_bass_kernel_spmd

F32 = mybir.dt.float32
F16 = mybir.dt.float16
AF = mybir.ActivationFunctionType
ALU = mybir.AluOpType

NCORES = 8
B, D, E, C, T = 65536, 512, 64, 10, 40
DEC, LR = 0.9, 0.1
NT = 512                      # batch columns per tile
BSH = B // NCORES             # 8192 batch rows per core
TILES = BSH // NT             # 16
PAIRS = TILES // 2            # 8
QUADS = PAIRS // 2            # 4

# Steady-state schedule per step (PE dense at 16 matmuls = 3.44us):
#   PE quads in order [0,1,2,3]. Each quad evac is split into its two
#   512-column pair halves running CONCURRENTLY: Scalar takes cols 0:512,
#   Vector takes cols 512:1024 — evac latency ~660ns, and subtile deps
#   give 8 independent pair-level chains, so every evac lands well before
#   its consumer matmul. S-accumulation: quad 0 on a Vector tensor_tensor
#   (appended after the Vector evac halves); quads 1-3 on the gpsimd
#   SW-DGE DMA queue with accum_op=add (3 x ~1us < step).
PE_ORDER = [0, 1, 2, 3]


def _bd(block):
    """[64,64] -> [128,128] block-diagonal (A rows 0:64, B rows 64:128)."""
    out = np.zeros((128, 128), np.float64)
    out[0:64, 0:64] = block
    out[64:128, 64:128] = block
    return out


def _host_prep(x, z0, W_feat, b_feat, W1, b1, W2, b2, class_emb):
    """Host-side prep: fp16 prescaled weight tables + fp16 input shards."""
    f8 = np.float64
    W1f = W1[:, :E].astype(f8)
    W1z = W1[:, E:2 * E].astype(f8)
    w1t = W1[:, 2 * E].astype(f8)
    W2_ = W2.astype(f8)
    CE_ = class_emb.astype(f8)

    G = (LR / DEC) * (W1z @ W2_)                    # [64,64]
    Wu = W1f @ W_feat.astype(f8)                    # [64,512]
    CL = LR * (CE_ @ W2_)                           # [10,64]
    CEi = DEC * CE_                                 # [10,64]

    wu = Wu.T.reshape(4, 128, E).transpose(1, 0, 2).reshape(128, 4 * E)
    g_bd = _bd(G.T)
    w1z_bd = _bd(W1z.T)
    sdiag = np.zeros((128, (T - 1) * 128), f8)
    for t in range(1, T):
        s = LR * DEC ** (-t)
        sdiag[:, (t - 1) * 128:t * 128] = s * np.eye(128)
    cl_bd = np.zeros((128, 128), f8)
    cl_bd[0:64, 0:C] = CL.T
    cl_bd[64:128, C:2 * C] = CL.T
    ce_bd = np.zeros((128, 128), f8)
    ce_bd[0:64, 0:C] = CEi.T
    ce_bd[64:128, C:2 * C] = CEi.T
    c16 = np.concatenate([wu, g_bd, w1z_bd, sdiag, cl_bd, ce_bd],
                         axis=1).astype(np.float16)

    beta = np.stack([
        DEC ** (-t) * (b1 + (t / T) * w1t + (1 - DEC ** t) * (W1z @ b2)
                       + W1f @ b_feat)
        for t in range(T)
    ]).T                                            # [64, T]
    beta = np.concatenate([beta, beta], axis=0)     # [128, T]
    biasl = np.zeros((128, 1), f8)
    bl = (1 - DEC ** T) * (CE_ @ b2)
    biasl[0:C, 0] = bl
    biasl[C:2 * C, 0] = bl
    c32 = np.concatenate([beta, biasl], axis=1).astype(np.float32)  # [128,T+1]

    # x -> per-core per-tile [128, 4*NT] fp16: x_dev[c,i,p,NT*k+n] = x[.,128k+p]
    xr = x.astype(np.float16).reshape(NCORES, TILES, NT, 4, 128)
    x_dev = np.ascontiguousarray(
        xr.transpose(0, 1, 4, 3, 2).reshape(NCORES, TILES, 128, 4 * NT))
    # z0 -> per-core per-pair [128, NT] fp16: rows 0:64 tile 2p, 64:128 2p+1
    zr = z0.astype(np.float16).reshape(NCORES, PAIRS, 2, NT, E)
    z0_dev = np.ascontiguousarray(
        zr.transpose(0, 1, 2, 4, 3).reshape(NCORES, PAIRS, 128, NT))

    return {"c16_d": c16, "c32_d": c32}, x_dev, z0_dev


def build(t_steps=T):
    nc = bacc.Bacc("TRN2", target_bir_lowering=False, debug=False)

    x_d = nc.dram_tensor("x_d", [TILES, 128, 4 * NT], F16,
                         kind="ExternalInput").ap()
    z0_d = nc.dram_tensor("z0_d", [PAIRS, 128, NT], F16,
                          kind="ExternalInput").ap()
    N16 = 4 * E + 128 + 128 + (T - 1) * 128 + 128 + 128
    c16_d = nc.dram_tensor("c16_d", [128, N16], F16, kind="ExternalInput").ap()
    c32_d = nc.dram_tensor("c32_d", [128, T + 1], F32,
                           kind="ExternalInput").ap()
    out_d = nc.dram_tensor("out_d", [PAIRS, 2, C, NT], F32,
                           kind="ExternalOutput").ap()

    scale_l = float(DEC ** (t_steps - 1))

    with tile.TileContext(nc) as tc:
        with (
            tc.sbuf_pool(name="c16", bufs=1) as c16pool,
            tc.sbuf_pool(name="c32", bufs=1) as c32pool,
            tc.sbuf_pool(name="xt", bufs=8) as xpool,
            tc.sbuf_pool(name="uu", bufs=QUADS) as upool,
            tc.sbuf_pool(name="z0s", bufs=PAIRS) as zpool,
            tc.sbuf_pool(name="ss", bufs=QUADS) as spool,
            tc.sbuf_pool(name="hh", bufs=3 * QUADS) as hpool,
            tc.sbuf_pool(name="ll", bufs=4) as lpool,
            tc.psum_pool(name="ha", bufs=QUADS) as hapool,
        ):
            c16 = c16pool.tile([128, N16], F16)
            nc.sync.dma_start(c16[:, 0:512], c16_d[:, 0:512])
            nc.sync.dma_start(c16[:, 512:N16], c16_d[:, 512:N16])
            c32 = c32pool.tile([128, T + 1], F32)
            nc.sync.dma_start(c32, c32_d)
            o = 0
            def _sl(n):
                nonlocal o
                v = c16[:, o:o + n]; o += n; return v
            wu_sb = _sl(4 * E); g_sb = _sl(128); w1z_sb = _sl(128)
            sd_sb = _sl((T - 1) * 128); cl_sb = _sl(128); ce_sb = _sl(128)
            beta_sb = c32[:, 0:T]; biasl_sb = c32[:, T:T + 1]

            EVAC_ENG = {0: 'v', 1: 'a', 2: 'a', 3: 'a'}

            def evac(q, dst, src, bias_ap):
                if EVAC_ENG[q] == 'a':
                    nc.scalar.activation(dst, src, AF.Relu, bias=bias_ap,
                                         scale=1.0)
                else:
                    nc.vector.tensor_scalar(dst, src, bias_ap, 0.0,
                                            ALU.add, ALU.max)

            def sacc(q, t, hh):
                if q in (0, 1):
                    nc.vector.tensor_tensor(SS[q], SS[q], hh, ALU.add)
                else:
                    dst = SS[q] if t % 2 == 0 else SSO[q]
                    nc.gpsimd.dma_start(dst, hh, accum_op=ALU.add)

            HA, HH, UU, SS, Z0 = {}, {}, {}, {}, {}
            # ---- feature phase: u = Wu@x accumulated straight into P ----
            for q in range(QUADS):
                ha = hapool.tile([128, 2 * NT], F32, tag="ha", name=f"ha{q}")
                HA[q] = ha
                xts = {}
                for r in range(2):
                    for ab in range(2):
                        p = 2 * q + r
                        xt = xpool.tile([128, 4 * NT], F16, tag="xt",
                                        name=f"xt{p}_{ab}")
                        nc.gpsimd.dma_start(xt, x_d[2 * p + ab])
                        xts[(r, ab)] = xt
                for r in range(2):          # pair index within quad
                    p = 2 * q + r
                    pcol = ha[:, NT * r:NT * (r + 1)]
                    for ab in range(2):
                        dst = pcol[0:64] if ab == 0 else pcol[64:128]
                        xt = xts[(r, ab)]
                        for k in range(4):
                            cols = slice(NT * k, NT * (k + 1))
                            nc.tensor.matmul(dst, wu_sb[:, E * k:E * (k + 1)],
                                             xt[:, cols],
                                             start=(k == 0), stop=(k == 3),
                                             skip_group_check=True)
                    z0t = zpool.tile([128, NT], F16, tag="z0s", name=f"z0t{p}")
                    nc.sync.dma_start(z0t, z0_d[p])
                    Z0[p] = z0t
                uu = upool.tile([128, 2 * NT], F16, tag="uu", name=f"uu{q}")
                nc.scalar.activation(uu, ha, AF.Copy, bias=0.0, scale=1.0)
                UU[q] = uu

            # ---- step 0: P_0 = u + W1z@z0 ----
            for q in PE_ORDER:
                for r in range(2):
                    nc.tensor.matmul(HA[q][:, NT * r:NT * (r + 1)], w1z_sb,
                                     Z0[2 * q + r], start=False, stop=False,
                                     skip_group_check=True)
            for q in PE_ORDER:
                hh = hpool.tile([128, 2 * NT], F16, tag=f"hh{q}", bufs=8, name=f"hh{q}_0")
                evac(q, hh, HA[q], beta_sb[:, 0:1])
                HH[q] = hh
            SSO = {}
            for q in PE_ORDER:
                ss = spool.tile([128, 2 * NT], F16, tag="ss", name=f"ss{q}")
                nc.vector.tensor_scalar_add(ss, HH[q], 0.0)
                SS[q] = ss
            for q in (2, 3):
                sso = spool.tile([128, 2 * NT], F16, tag="sso", bufs=2,
                                 name=f"sso{q}")
                nc.vector.memset(sso, 0.0)
                SSO[q] = sso

            # ---- steps 1..T-1 ----
            for t in range(1, t_steps):
                sd = sd_sb[:, 128 * (t - 1):128 * t]
                last = t == t_steps - 1
                for q in PE_ORDER:
                    for r in range(2):
                        cols = slice(NT * r, NT * (r + 1))
                        nc.tensor.matmul(HA[q][:, cols], g_sb, HH[q][:, cols],
                                         start=False, stop=False,
                                         skip_group_check=True)
                        nc.tensor.matmul(HA[q][:, cols], sd, UU[q][:, cols],
                                         start=False, stop=last,
                                         skip_group_check=True)
                for q in PE_ORDER:
                    hh = hpool.tile([128, 2 * NT], F16, tag=f"hh{q}", bufs=8,
                                    name=f"hh{q}_{t}")
                    evac(q, hh, HA[q], beta_sb[:, t:t + 1])
                    HH[q] = hh
                    if q == 1:
                        sacc(0, t, HH[0])
                    elif q == 2:
                        sacc(1, t, HH[1])
                for q in (2, 3):
                    sacc(q, t, HH[q])

            # ---- final: L = BD(0.9*CE)@z0 + BD(CL)@S; logits evac + store ----
            for q in range(QUADS):
                lb = hapool.tile([128, 2 * NT], F32, tag="ha", name=f"lb{q}")
                for r in range(2):
                    cols = slice(NT * r, NT * (r + 1))
                    nc.tensor.matmul(lb[:, cols], ce_sb, Z0[2 * q + r],
                                     start=True, stop=False,
                                     skip_group_check=True)
                    nc.tensor.matmul(lb[:, cols], cl_sb, SS[q][:, cols],
                                     start=False, stop=(q < 2),
                                     skip_group_check=True)
                    if q >= 2:
                        nc.tensor.matmul(lb[:, cols], cl_sb, SSO[q][:, cols],
                                         start=False, stop=True,
                                         skip_group_check=True)
                ll = lpool.tile([128, 2 * NT], F32, tag="ll", name=f"ll{q}")
                if q % 2 == 0:
                    nc.scalar.activation(ll, lb, AF.Identity, bias=biasl_sb,
                                         scale=scale_l)
                else:
                    nc.vector.tensor_scalar(ll, lb, scale_l, biasl_sb,
                                            ALU.mult, ALU.add)
                for r in range(2):
                    cols = slice(NT * r, NT * (r + 1))
                    eng = nc.sync if r == 0 else nc.scalar
                    eng.dma_start(out_d[2 * q + r], ll[0:2 * C, cols])
    nc.compile()
    return nc


_BUILT = {}


def _get_nc(t_steps=T):
    if t_steps not in _BUILT:
        _BUILT[t_steps] = build(t_steps)
    return _BUILT[t_steps]


def kernel(x, z0, W_feat, b_feat, W1, b1, W2, b2, class_emb, T_steps, **run_kw):
    x = np.asarray(x); z0 = np.asarray(z0)
    assert int(T_steps) == T
    const, x_dev, z0_dev = _host_prep(
        x, z0, np.asarray(W_feat), np.asarray(b_feat),
        np.asarray(W1), np.asarray(b1), np.asarray(W2), np.asarray(b2),
        np.asarray(class_emb))
    nc = _get_nc()
    in_maps = []
    for c in range(NCORES):
        m = dict(const)
        m["x_d"] = x_dev[c]
        m["z0_d"] = z0_dev[c]
        in_maps.append(m)
    res = run_bass_kernel_spmd(nc, in_maps, core_ids=list(range(NCORES)), **run_kw)
    outs = [r["out_d"] for r in res.results]       # each [PAIRS, 2, C, NT]
    stacked = np.stack(outs)                       # [8, 8, 2, 10, 512]
    logits = stacked.transpose(0, 1, 2, 4, 3).reshape(B, C)
    if run_kw:
        kernel.last_result = res
    return np.ascontiguousarray(logits.astype(np.float32))


# revision 25
# speedup vs baseline: 1.0103x; 1.0103x over previous
"""Trainium2 Bass kernel for nn_IterativeClassifier (B=65536, D=512, E=64, C=10, T=40).

Pure data parallel over 8 cores (batch-sharded). Per core, activations live
TRANSPOSED on-chip: [E, batch] with batch on the free dim; two batch tiles of
512 are paired onto 128 partitions (tile A rows 0:64, B rows 64:128), and two
pairs share one [128,1024] 2-bank PSUM tile (a "quad", 4 quads = all 8 banks).

Using relu positive-homogeneity and the de-scaled substitution
h^_t := 0.9^-t h_t, the whole 40-step loop runs on ONE persistent PSUM
accumulator P per pair that is never restarted:
    P_0 = u + W1z@z0          u := (W1f@W_feat) @ x   (fused on host)
    P_t = P_{t-1} + G'@h^_{t-1} + s_t*u     G' = (0.1/0.9) W1z@W2,
                                            s_t = 0.1*0.9^-t
    h^_t = relu(P_t + beta_t)               (evac, PSUM -> fp16 SBUF)
All matmuls take fp16 inputs (1 cy/row on the PE, vs 4 for fp32) with
128-wide block-diagonal stationaries, so a step is exactly TWO N=512 matmul
instructions per pair = 16 per step per core (~3.44us, PE-dense).

The logits matmul stays OFF the PE during the loop: S := sum_t h^_t is
accumulated in fp16 and at the end
    logits = 0.9^39 * (BD(0.1*CE@W2)@S + BD(0.9*CE)@z0) + biasL.
Per-step engine schedule (all under the PE's 3.44us):
  - evacs: Vector takes quad 0 (first in PE order), Scalar quads 1-3;
  - S-accumulate: quads 0,1 as Vector tensor_tensor (interleaved between
    the Vector evac and the Scalar evacs' consumers), quads 2,3 on the
    gpsimd SW-DGE DMA queue with accum_op=add, double-buffered into
    even/odd accumulators so the per-tile DMA WAW chain (~2.4us
    latency/hop) spans two steps and never backs up.
x is shipped as fp16 [128, 2048] tiles (8 MB/core, DMA-bound feature phase)
with 8 tiles of prefetch depth; hh tiles are 8-deep per quad so evac
slot-reuse never waits on the (lagging) S-DMA queue.
"""

import numpy as np

import concourse.bass as bass
import concourse.bacc as bacc
import concourse.mybir as mybir
import concourse.tile as tile
from concourse.bass_utils import run_bass_kernel_spmd

F32 = mybir.dt.float32
F16 = mybir.dt.float16
AF = mybir.ActivationFunctionType
ALU = mybir.AluOpType

NCORES = 8
B, D, E, C, T = 65536, 512, 64, 10, 40
DEC, LR = 0.9, 0.1
NT = 512                      # batch columns per tile
BSH = B // NCORES             # 8192 batch rows per core
TILES = BSH // NT             # 16
PAIRS = TILES // 2            # 8
QUADS = PAIRS // 2            # 4

# Steady-state schedule per step (PE dense at 16 matmuls = 3.44us):
#   PE quads in order [0,1,2,3]. Each quad evac is split into its two
#   512-column pair halves running CONCURRENTLY: Scalar takes cols 0:512,
#   Vector takes cols 512:1024 — evac latency ~660ns, and subtile deps
#   give 8 independent pair-level chains, so every evac lands well before
#   its consumer matmul. S-accumulation: quad 0 on a Vector tensor_tensor
#   (appended after the Vector evac halves); quads 1-3 on the gpsimd
#   SW-DGE DMA queue with accum_op=add (3 x ~1us < step).
PE_ORDER = [0, 1, 2, 3]


def _bd(block):
    """[64,64] -> [128,128] block-diagonal (A rows 0:64, B rows 64:128)."""
    out = np.zeros((128, 128), np.float64)
    out[0:64, 0:64] = block
    out[64:128, 64:128] = block
    return out


def _host_prep(x, z0, W_feat, b_feat, W1, b1, W2, b2, class_emb):
    """Host-side prep: fp16 prescaled weight tables + fp16 input shards."""
    f8 = np.float64
    W1f = W1[:, :E].astype(f8)
    W1z = W1[:, E:2 * E].astype(f8)
    w1t = W1[:, 2 * E].astype(f8)
    W2_ = W2.astype(f8)
    CE_ = class_emb.astype(f8)

    G = (LR / DEC) * (W1z @ W2_)                    # [64,64]
    Wu = W1f @ W_feat.astype(f8)                    # [64,512]
    CL = LR * (CE_ @ W2_)                           # [10,64]
    CEi = DEC * CE_                                 # [10,64]

    wu = Wu.T.reshape(4, 128, E).transpose(1, 0, 2).reshape(128, 4 * E)
    g_bd = _bd(G.T)
    w1z_bd = _bd(W1z.T)
    sdiag = np.zeros((128, (T - 1) * 128), f8)
    for t in range(1, T):
        s = LR * DEC ** (-t)
        sdiag[:, (t - 1) * 128:t * 128] = s * np.eye(128)
    cl_bd = np.zeros((128, 128), f8)
    cl_bd[0:64, 0:C] = CL.T
    cl_bd[64:128, C:2 * C] = CL.T
    ce_bd = np.zeros((128, 128), f8)
    ce_bd[0:64, 0:C] = CEi.T
    ce_bd[64:128, C:2 * C] = CEi.T
    c16 = np.concatenate([wu, g_bd, w1z_bd, sdiag, cl_bd, ce_bd],
                         axis=1).astype(np.float16)

    beta = np.stack([
        DEC ** (-t) * (b1 + (t / T) * w1t + (1 - DEC ** t) * (W1z @ b2)
                       + W1f @ b_feat)
        for t in range(T)
    ]).T                                            # [64, T]
    beta = np.concatenate([beta, beta], axis=0)     # [128, T]
    biasl = np.zeros((128, 1), f8)
    bl = (1 - DEC ** T) * (CE_ @ b2)
    biasl[0:C, 0] = bl
    biasl[C:2 * C, 0] = bl
    c32 = np.concatenate([beta, biasl], axis=1).astype(np.float32)  # [128,T+1]

    # x -> per-core per-tile [128, 4*NT] fp16: x_dev[c,i,p,NT*k+n] = x[.,128k+p]
    xr = x.astype(np.float16).reshape(NCORES, TILES, NT, 4, 128)
    x_dev = np.ascontiguousarray(
        xr.transpose(0, 1, 4, 3, 2).reshape(NCORES, TILES, 128, 4 * NT))
    # z0 -> per-core per-pair [128, NT] fp16: rows 0:64 tile 2p, 64:128 2p+1
    zr = z0.astype(np.float16).reshape(NCORES, PAIRS, 2, NT, E)
    z0_dev = np.ascontiguousarray(
        zr.transpose(0, 1, 2, 4, 3).reshape(NCORES, PAIRS, 128, NT))

    return {"c16_d": c16, "c32_d": c32}, x_dev, z0_dev


def build(t_steps=T):
    nc = bacc.Bacc("TRN2", target_bir_lowering=False, debug=False)

    x_d = nc.dram_tensor("x_d", [TILES, 128, 4 * NT], F16,
                         kind="ExternalInput").ap()
    z0_d = nc.dram_tensor("z0_d", [PAIRS, 128, NT], F16,
                          kind="ExternalInput").ap()
    N16 = 4 * E + 128 + 128 + (T - 1) * 128 + 128 + 128
    c16_d = nc.dram_tensor("c16_d", [128, N16], F16, kind="ExternalInput").ap()
    c32_d = nc.dram_tensor("c32_d", [128, T + 1], F32,
                           kind="ExternalInput").ap()
    out_d = nc.dram_tensor("out_d", [PAIRS, 2, C, NT], F32,
                           kind="ExternalOutput").ap()

    scale_l = float(DEC ** (t_steps - 1))

    with tile.TileContext(nc) as tc:
        with (
            tc.sbuf_pool(name="c16", bufs=1) as c16pool,
            tc.sbuf_pool(name="c32", bufs=1) as c32pool,
            tc.sbuf_pool(name="xt", bufs=8) as xpool,
            tc.sbuf_pool(name="uu", bufs=QUADS) as upool,
            tc.sbuf_pool(name="z0s", bufs=PAIRS) as zpool,
            tc.sbuf_pool(name="ss", bufs=QUADS) as spool,
            tc.sbuf_pool(name="hh", bufs=3 * QUADS) as hpool,
            tc.sbuf_pool(name="ll", bufs=4) as lpool,
            tc.psum_pool(name="ha", bufs=QUADS) as hapool,
        ):
            c16 = c16pool.tile([128, N16], F16)
            nc.sync.dma_start(c16[:, 0:512], c16_d[:, 0:512])
            c32 = c32pool.tile([128, T + 1], F32)
            nc.sync.dma_start(c32, c32_d)
            o = 0
            def _sl(n):
                nonlocal o
                v = c16[:, o:o + n]; o += n; return v
            wu_sb = _sl(4 * E); g_sb = _sl(128); w1z_sb = _sl(128)
            sd_sb = _sl((T - 1) * 128); cl_sb = _sl(128); ce_sb = _sl(128)
            beta_sb = c32[:, 0:T]; biasl_sb = c32[:, T:T + 1]

            EVAC_ENG = {0: 'v', 1: 'a', 2: 'a', 3: 'a'}

            def evac(q, dst, src, bias_ap):
                if EVAC_ENG[q] == 'a':
                    nc.scalar.activation(dst, src, AF.Relu, bias=bias_ap,
                                         scale=1.0)
                else:
                    nc.vector.tensor_scalar(dst, src, bias_ap, 0.0,
                                            ALU.add, ALU.max)

            def sacc(q, t, hh):
                if q in (0, 1):
                    nc.vector.tensor_tensor(SS[q], SS[q], hh, ALU.add)
                else:
                    dst = SS[q] if t % 2 == 0 else SSO[q]
                    nc.gpsimd.dma_start(dst, hh, accum_op=ALU.add)

            HA, HH, UU, SS, Z0 = {}, {}, {}, {}, {}
            SSO = {}
            for q in (2, 3):
                sso = spool.tile([128, 2 * NT], F16, tag="sso", bufs=2,
                                 name=f"sso{q}")
                nc.vector.memset(sso, 0.0)
                SSO[q] = sso
            # ---- feature phase: u = Wu@x accumulated straight into P ----
            for q in range(QUADS):
                ha = hapool.tile([128, 2 * NT], F32, tag="ha", name=f"ha{q}")
                HA[q] = ha
                xts = {}
                for r in range(2):
                    for ab in range(2):
                        p = 2 * q + r
                        xt = xpool.tile([128, 4 * NT], F16, tag="xt",
                                        name=f"xt{p}_{ab}")
                        eng = nc.gpsimd if ab == 0 else nc.sync
                        eng.dma_start(xt, x_d[2 * p + ab])
                        xts[(r, ab)] = xt
                for r in range(2):          # pair index within quad
                    p = 2 * q + r
                    pcol = ha[:, NT * r:NT * (r + 1)]
                    for ab in range(2):
                        dst = pcol[0:64] if ab == 0 else pcol[64:128]
                        xt = xts[(r, ab)]
                        for k in range(4):
                            cols = slice(NT * k, NT * (k + 1))
                            nc.tensor.matmul(dst, wu_sb[:, E * k:E * (k + 1)],
                                             xt[:, cols],
                                             start=(k == 0), stop=(k == 3),
                                             skip_group_check=True)
                    z0t = zpool.tile([128, NT], F16, tag="z0s", name=f"z0t{p}")
                    nc.sync.dma_start(z0t, z0_d[p])
                    Z0[p] = z0t
                uu = upool.tile([128, 2 * NT], F16, tag="uu", name=f"uu{q}")
                nc.scalar.activation(uu, ha, AF.Copy, bias=0.0, scale=1.0)
                UU[q] = uu

            nc.sync.dma_start(c16[:, 512:N16], c16_d[:, 512:N16])

            # ---- step 0: P_0 = u + W1z@z0 ----
            for q in PE_ORDER:
                for r in range(2):
                    nc.tensor.matmul(HA[q][:, NT * r:NT * (r + 1)], w1z_sb,
                                     Z0[2 * q + r], start=False, stop=False,
                                     skip_group_check=True)
            for q in PE_ORDER:
                hh = hpool.tile([128, 2 * NT], F16, tag=f"hh{q}", bufs=8, name=f"hh{q}_0")
                evac(q, hh, HA[q], beta_sb[:, 0:1])
                HH[q] = hh
            for q in PE_ORDER:
                ss = spool.tile([128, 2 * NT], F16, tag="ss", name=f"ss{q}")
                nc.vector.tensor_scalar_add(ss, HH[q], 0.0)
                SS[q] = ss

            # ---- steps 1..T-1 ----
            for t in range(1, t_steps):
                sd = sd_sb[:, 128 * (t - 1):128 * t]
                last = t == t_steps - 1
                for q in PE_ORDER:
                    for r in range(2):
                        cols = slice(NT * r, NT * (r + 1))
                        nc.tensor.matmul(HA[q][:, cols], g_sb, HH[q][:, cols],
                                         start=False, stop=False,
                                         skip_group_check=True)
                        nc.tensor.matmul(HA[q][:, cols], sd, UU[q][:, cols],
                                         start=False, stop=last,
                                         skip_group_check=True)
                for q in PE_ORDER:
                    hh = hpool.tile([128, 2 * NT], F16, tag=f"hh{q}", bufs=8,
                                    name=f"hh{q}_{t}")
                    evac(q, hh, HA[q], beta_sb[:, t:t + 1])
                    HH[q] = hh
                    if q == 1:
                        sacc(0, t, HH[0])
                    elif q == 2:
                        sacc(1, t, HH[1])
                for q in (2, 3):
                    sacc(q, t, HH[q])

            # ---- final: L = BD(0.9*CE)@z0 + BD(CL)@S; logits evac + store ----
            for q in range(QUADS):
                lb = hapool.tile([128, 2 * NT], F32, tag="ha", name=f"lb{q}")
                for r in range(2):
                    cols = slice(NT * r, NT * (r + 1))
                    nc.tensor.matmul(lb[:, cols], ce_sb, Z0[2 * q + r],
                                     start=True, stop=False,
                                     skip_group_check=True)
                    nc.tensor.matmul(lb[:, cols], cl_sb, SS[q][:, cols],
                                     start=False, stop=(q < 2),
                                     skip_group_check=True)
                    if q >= 2:
                        nc.tensor.matmul(lb[:, cols], cl_sb, SSO[q][:, cols],
                                         start=False, stop=True,
                                         skip_group_check=True)
                ll = lpool.tile([128, 2 * NT], F32, tag="ll", name=f"ll{q}")
                if q % 2 == 0:
                    nc.scalar.activation(ll, lb, AF.Identity, bias=biasl_sb,
                                         scale=scale_l)
                else:
                    nc.vector.tensor_scalar(ll, lb, scale_l, biasl_sb,
                                            ALU.mult, ALU.add)
                for r in range(2):
                    cols = slice(NT * r, NT * (r + 1))
                    eng = nc.sync if r == 0 else nc.scalar
                    eng.dma_start(out_d[2 * q + r], ll[0:2 * C, cols])
    nc.compile()
    return nc


_BUILT = {}


def _get_nc(t_steps=T):
    if t_steps not in _BUILT:
        _BUILT[t_steps] = build(t_steps)
    return _BUILT[t_steps]


def kernel(x, z0, W_feat, b_feat, W1, b1, W2, b2, class_emb, T_steps, **run_kw):
    x = np.asarray(x); z0 = np.asarray(z0)
    assert int(T_steps) == T
    const, x_dev, z0_dev = _host_prep(
        x, z0, np.asarray(W_feat), np.asarray(b_feat),
        np.asarray(W1), np.asarray(b1), np.asarray(W2), np.asarray(b2),
        np.asarray(class_emb))
    nc = _get_nc()
    in_maps = []
    for c in range(NCORES):
        m = dict(const)
        m["x_d"] = x_dev[c]
        m["z0_d"] = z0_dev[c]
        in_maps.append(m)
    res = run_bass_kernel_spmd(nc, in_maps, core_ids=list(range(NCORES)), **run_kw)
    outs = [r["out_d"] for r in res.results]       # each [PAIRS, 2, C, NT]
    stacked = np.stack(outs)                       # [8, 8, 2, 10, 512]
    logits = stacked.transpose(0, 1, 2, 4, 3).reshape(B, C)
    if run_kw:
        kernel.last_result = res
    return np.ascontiguousarray(logits.astype(np.float32))


# revision 26
# speedup vs baseline: 1.0231x; 1.0127x over previous
"""Trainium2 Bass kernel for nn_IterativeClassifier (B=65536, D=512, E=64, C=10, T=40).

Pure data parallel over 8 cores (batch-sharded). Per core, activations live
TRANSPOSED on-chip: [E, batch] with batch on the free dim; two batch tiles of
512 are paired onto 128 partitions (tile A rows 0:64, B rows 64:128), and two
pairs share one [128,1024] 2-bank PSUM tile (a "quad", 4 quads = all 8 banks).

Using relu positive-homogeneity and the de-scaled substitution
h^_t := 0.9^-t h_t, the whole 40-step loop runs on ONE persistent PSUM
accumulator P per pair that is never restarted:
    P_0 = u + W1z@z0          u := (W1f@W_feat) @ x   (fused on host)
    P_t = P_{t-1} + G'@h^_{t-1} + s_t*u     G' = (0.1/0.9) W1z@W2,
                                            s_t = 0.1*0.9^-t
    h^_t = relu(P_t + beta_t)               (evac, PSUM -> fp16 SBUF)
All matmuls take fp16 inputs (1 cy/row on the PE, vs 4 for fp32) with
128-wide block-diagonal stationaries, so a step is exactly TWO N=512 matmul
instructions per pair = 16 per step per core (~3.44us, PE-dense).

The logits matmul stays OFF the PE during the loop: S := sum_t h^_t is
accumulated in fp16 and at the end
    logits = 0.9^39 * (BD(0.1*CE@W2)@S + BD(0.9*CE)@z0) + biasL.
Per-step engine schedule (all under the PE's 3.44us):
  - evacs: Vector takes quad 0 (first in PE order), Scalar quads 1-3;
  - S-accumulate: quads 0,1 as Vector tensor_tensor (interleaved between
    the Vector evac and the Scalar evacs' consumers), quads 2,3 on the
    gpsimd SW-DGE DMA queue with accum_op=add, double-buffered into
    even/odd accumulators so the per-tile DMA WAW chain (~2.4us
    latency/hop) spans two steps and never backs up.
x is shipped as fp16 [128, 2048] tiles (8 MB/core, DMA-bound feature phase)
with 8 tiles of prefetch depth; hh tiles are 8-deep per quad so evac
slot-reuse never waits on the (lagging) S-DMA queue.
"""

import numpy as np

import concourse.bass as bass
import concourse.bacc as bacc
import concourse.mybir as mybir
import concourse.tile as tile
from concourse.bass_utils import run_bass_kernel_spmd

F32 = mybir.dt.float32
F16 = mybir.dt.float16
AF = mybir.ActivationFunctionType
ALU = mybir.AluOpType

NCORES = 8
B, D, E, C, T = 65536, 512, 64, 10, 40
DEC, LR = 0.9, 0.1
NT = 512                      # batch columns per tile
BSH = B // NCORES             # 8192 batch rows per core
TILES = BSH // NT             # 16
PAIRS = TILES // 2            # 8
QUADS = PAIRS // 2            # 4

# Steady-state schedule per step (PE dense at 16 matmuls = 3.44us):
#   PE quads in order [0,1,2,3]. Each quad evac is split into its two
#   512-column pair halves running CONCURRENTLY: Scalar takes cols 0:512,
#   Vector takes cols 512:1024 — evac latency ~660ns, and subtile deps
#   give 8 independent pair-level chains, so every evac lands well before
#   its consumer matmul. S-accumulation: quad 0 on a Vector tensor_tensor
#   (appended after the Vector evac halves); quads 1-3 on the gpsimd
#   SW-DGE DMA queue with accum_op=add (3 x ~1us < step).
PE_ORDER = [0, 1, 2, 3]


def _bd(block):
    """[64,64] -> [128,128] block-diagonal (A rows 0:64, B rows 64:128)."""
    out = np.zeros((128, 128), np.float64)
    out[0:64, 0:64] = block
    out[64:128, 64:128] = block
    return out


def _host_prep(x, z0, W_feat, b_feat, W1, b1, W2, b2, class_emb):
    """Host-side prep: fp16 prescaled weight tables + fp16 input shards."""
    f8 = np.float64
    W1f = W1[:, :E].astype(f8)
    W1z = W1[:, E:2 * E].astype(f8)
    w1t = W1[:, 2 * E].astype(f8)
    W2_ = W2.astype(f8)
    CE_ = class_emb.astype(f8)

    G = (LR / DEC) * (W1z @ W2_)                    # [64,64]
    Wu = W1f @ W_feat.astype(f8)                    # [64,512]
    CL = LR * (CE_ @ W2_)                           # [10,64]
    CEi = DEC * CE_                                 # [10,64]

    wu = Wu.T.reshape(4, 128, E).transpose(1, 0, 2).reshape(128, 4 * E)
    g_bd = _bd(G.T)
    w1z_bd = _bd(W1z.T)
    sdiag = np.zeros((128, (T - 1) * 128), f8)
    for t in range(1, T):
        s = LR * DEC ** (-t)
        sdiag[:, (t - 1) * 128:t * 128] = s * np.eye(128)
    cl_bd = np.zeros((128, 128), f8)
    cl_bd[0:64, 0:C] = CL.T
    cl_bd[64:128, C:2 * C] = CL.T
    ce_bd = np.zeros((128, 128), f8)
    ce_bd[0:64, 0:C] = CEi.T
    ce_bd[64:128, C:2 * C] = CEi.T
    c16 = np.concatenate([wu, g_bd, w1z_bd, sdiag, cl_bd, ce_bd],
                         axis=1).astype(np.float16)

    beta = np.stack([
        DEC ** (-t) * (b1 + (t / T) * w1t + (1 - DEC ** t) * (W1z @ b2)
                       + W1f @ b_feat)
        for t in range(T)
    ]).T                                            # [64, T]
    beta = np.concatenate([beta, beta], axis=0)     # [128, T]
    biasl = np.zeros((128, 1), f8)
    bl = (1 - DEC ** T) * (CE_ @ b2)
    biasl[0:C, 0] = bl
    biasl[C:2 * C, 0] = bl
    c32 = np.concatenate([beta, biasl], axis=1).astype(np.float32)  # [128,T+1]

    # x -> per-core per-tile [128, 4*NT] fp16: x_dev[c,i,p,NT*k+n] = x[.,128k+p]
    xr = x.astype(np.float16).reshape(NCORES, TILES, NT, 4, 128)
    x_dev = np.ascontiguousarray(
        xr.transpose(0, 1, 4, 3, 2).reshape(NCORES, TILES, 128, 4 * NT))
    # z0 -> per-core per-pair [128, NT] fp16: rows 0:64 tile 2p, 64:128 2p+1
    zr = z0.astype(np.float16).reshape(NCORES, PAIRS, 2, NT, E)
    z0_dev = np.ascontiguousarray(
        zr.transpose(0, 1, 2, 4, 3).reshape(NCORES, PAIRS, 128, NT))

    return {"c16_d": c16, "c32_d": c32}, x_dev, z0_dev


def build(t_steps=T):
    nc = bacc.Bacc("TRN2", target_bir_lowering=False, debug=False)

    x_d = nc.dram_tensor("x_d", [TILES, 128, 4 * NT], F16,
                         kind="ExternalInput").ap()
    z0_d = nc.dram_tensor("z0_d", [PAIRS, 128, NT], F16,
                          kind="ExternalInput").ap()
    N16 = 4 * E + 128 + 128 + (T - 1) * 128 + 128 + 128
    c16_d = nc.dram_tensor("c16_d", [128, N16], F16, kind="ExternalInput").ap()
    c32_d = nc.dram_tensor("c32_d", [128, T + 1], F32,
                           kind="ExternalInput").ap()
    out_d = nc.dram_tensor("out_d", [PAIRS, 2, C, NT], F32,
                           kind="ExternalOutput").ap()

    scale_l = float(DEC ** (t_steps - 1))

    with tile.TileContext(nc) as tc:
        with (
            tc.sbuf_pool(name="c16", bufs=1) as c16pool,
            tc.sbuf_pool(name="c32", bufs=1) as c32pool,
            tc.sbuf_pool(name="xt", bufs=8) as xpool,
            tc.sbuf_pool(name="uu", bufs=QUADS) as upool,
            tc.sbuf_pool(name="z0s", bufs=PAIRS) as zpool,
            tc.sbuf_pool(name="ss", bufs=QUADS) as spool,
            tc.sbuf_pool(name="hh", bufs=3 * QUADS) as hpool,
            tc.sbuf_pool(name="ll", bufs=4) as lpool,
            tc.psum_pool(name="ha", bufs=QUADS) as hapool,
        ):
            c16 = c16pool.tile([128, N16], F16)
            nc.sync.dma_start(c16[:, 0:512], c16_d[:, 0:512])
            c32 = c32pool.tile([128, T + 1], F32)
            nc.sync.dma_start(c32, c32_d)
            o = 0
            def _sl(n):
                nonlocal o
                v = c16[:, o:o + n]; o += n; return v
            wu_sb = _sl(4 * E); g_sb = _sl(128); w1z_sb = _sl(128)
            sd_sb = _sl((T - 1) * 128); cl_sb = _sl(128); ce_sb = _sl(128)
            beta_sb = c32[:, 0:T]; biasl_sb = c32[:, T:T + 1]

            EVAC_ENG = {0: 'v', 1: 'a', 2: 'a', 3: 'a'}

            def evac(q, dst, src, bias_ap):
                if EVAC_ENG[q] == 'a':
                    nc.scalar.activation(dst, src, AF.Relu, bias=bias_ap,
                                         scale=1.0)
                else:
                    nc.vector.tensor_scalar(dst, src, bias_ap, 0.0,
                                            ALU.add, ALU.max)

            def sacc(q, t, hh):
                if q in (0, 1):
                    nc.vector.tensor_tensor(SS[q], SS[q], hh, ALU.add)
                elif t == t_steps - 1:
                    # last step: Vector is about to idle and the serial
                    # DMA-accum latency (~2.4us) would sit on the tail chain
                    dst = SS[q] if t % 2 == 0 else SSO[q]
                    nc.vector.tensor_tensor(dst, dst, hh, ALU.add)
                else:
                    dst = SS[q] if t % 2 == 0 else SSO[q]
                    nc.gpsimd.dma_start(dst, hh, accum_op=ALU.add)

            HA, HH, UU, SS, Z0 = {}, {}, {}, {}, {}
            SSO = {}
            for q in (2, 3):
                sso = spool.tile([128, 2 * NT], F16, tag="sso", bufs=2,
                                 name=f"sso{q}")
                nc.vector.memset(sso, 0.0)
                SSO[q] = sso
            # ---- feature phase: u = Wu@x accumulated straight into P ----
            for q in range(QUADS):
                ha = hapool.tile([128, 2 * NT], F32, tag="ha", name=f"ha{q}")
                HA[q] = ha
                xts = {}
                for r in range(2):
                    for ab in range(2):
                        p = 2 * q + r
                        xt = xpool.tile([128, 4 * NT], F16, tag="xt",
                                        name=f"xt{p}_{ab}")
                        eng = nc.gpsimd if ab == 0 else nc.sync
                        eng.dma_start(xt, x_d[2 * p + ab])
                        xts[(r, ab)] = xt
                for r in range(2):          # pair index within quad
                    p = 2 * q + r
                    pcol = ha[:, NT * r:NT * (r + 1)]
                    for ab in range(2):
                        dst = pcol[0:64] if ab == 0 else pcol[64:128]
                        xt = xts[(r, ab)]
                        for k in range(4):
                            cols = slice(NT * k, NT * (k + 1))
                            nc.tensor.matmul(dst, wu_sb[:, E * k:E * (k + 1)],
                                             xt[:, cols],
                                             start=(k == 0), stop=(k == 3),
                                             skip_group_check=True)
                    z0t = zpool.tile([128, NT], F16, tag="z0s", name=f"z0t{p}")
                    nc.sync.dma_start(z0t, z0_d[p])
                    Z0[p] = z0t
                uu = upool.tile([128, 2 * NT], F16, tag="uu", name=f"uu{q}")
                nc.scalar.activation(uu, ha, AF.Copy, bias=0.0, scale=1.0)
                UU[q] = uu

            nc.sync.dma_start(c16[:, 512:N16], c16_d[:, 512:N16])

            # ---- step 0: P_0 = u + W1z@z0 ----
            for q in PE_ORDER:
                for r in range(2):
                    nc.tensor.matmul(HA[q][:, NT * r:NT * (r + 1)], w1z_sb,
                                     Z0[2 * q + r], start=False, stop=False,
                                     skip_group_check=True)
            for q in PE_ORDER:
                hh = hpool.tile([128, 2 * NT], F16, tag=f"hh{q}", bufs=8, name=f"hh{q}_0")
                evac(q, hh, HA[q], beta_sb[:, 0:1])
                HH[q] = hh
            for q in PE_ORDER:
                ss = spool.tile([128, 2 * NT], F16, tag="ss", name=f"ss{q}")
                nc.vector.tensor_scalar_add(ss, HH[q], 0.0)
                SS[q] = ss

            # ---- steps 1..T-1 ----
            for t in range(1, t_steps):
                sd = sd_sb[:, 128 * (t - 1):128 * t]
                last = t == t_steps - 1
                for q in PE_ORDER:
                    for r in range(2):
                        cols = slice(NT * r, NT * (r + 1))
                        nc.tensor.matmul(HA[q][:, cols], g_sb, HH[q][:, cols],
                                         start=False, stop=False,
                                         skip_group_check=True)
                        nc.tensor.matmul(HA[q][:, cols], sd, UU[q][:, cols],
                                         start=False, stop=last,
                                         skip_group_check=True)
                for q in PE_ORDER:
                    hh = hpool.tile([128, 2 * NT], F16, tag=f"hh{q}", bufs=8,
                                    name=f"hh{q}_{t}")
                    evac(q, hh, HA[q], beta_sb[:, t:t + 1])
                    HH[q] = hh
                    if q == 1:
                        sacc(0, t, HH[0])
                    elif q == 2:
                        sacc(1, t, HH[1])
                for q in (2, 3):
                    sacc(q, t, HH[q])

            # ---- final: L = BD(0.9*CE)@z0 + BD(CL)@S; logits evac + store ----
            for q in range(QUADS):
                lb = hapool.tile([128, 2 * NT], F32, tag="ha", name=f"lb{q}")
                for r in range(2):
                    cols = slice(NT * r, NT * (r + 1))
                    nc.tensor.matmul(lb[:, cols], ce_sb, Z0[2 * q + r],
                                     start=True, stop=False,
                                     skip_group_check=True)
                    nc.tensor.matmul(lb[:, cols], cl_sb, SS[q][:, cols],
                                     start=False, stop=(q < 2),
                                     skip_group_check=True)
                    if q >= 2:
                        nc.tensor.matmul(lb[:, cols], cl_sb, SSO[q][:, cols],
                                         start=False, stop=True,
                                         skip_group_check=True)
                ll = lpool.tile([128, 2 * NT], F32, tag="ll", name=f"ll{q}")
                if q % 2 == 0:
                    nc.scalar.activation(ll, lb, AF.Identity, bias=biasl_sb,
                                         scale=scale_l)
                else:
                    nc.vector.tensor_scalar(ll, lb, scale_l, biasl_sb,
                                            ALU.mult, ALU.add)
                for r in range(2):
                    cols = slice(NT * r, NT * (r + 1))
                    eng = nc.sync if r == 0 else nc.scalar
                    eng.dma_start(out_d[2 * q + r], ll[0:2 * C, cols])
    nc.compile()
    return nc


_BUILT = {}


def _get_nc(t_steps=T):
    if t_steps not in _BUILT:
        _BUILT[t_steps] = build(t_steps)
    return _BUILT[t_steps]


def kernel(x, z0, W_feat, b_feat, W1, b1, W2, b2, class_emb, T_steps, **run_kw):
    x = np.asarray(x); z0 = np.asarray(z0)
    assert int(T_steps) == T
    const, x_dev, z0_dev = _host_prep(
        x, z0, np.asarray(W_feat), np.asarray(b_feat),
        np.asarray(W1), np.asarray(b1), np.asarray(W2), np.asarray(b2),
        np.asarray(class_emb))
    nc = _get_nc()
    in_maps = []
    for c in range(NCORES):
        m = dict(const)
        m["x_d"] = x_dev[c]
        m["z0_d"] = z0_dev[c]
        in_maps.append(m)
    res = run_bass_kernel_spmd(nc, in_maps, core_ids=list(range(NCORES)), **run_kw)
    outs = [r["out_d"] for r in res.results]       # each [PAIRS, 2, C, NT]
    stacked = np.stack(outs)                       # [8, 8, 2, 10, 512]
    logits = stacked.transpose(0, 1, 2, 4, 3).reshape(B, C)
    if run_kw:
        kernel.last_result = res
    return np.ascontiguousarray(logits.astype(np.float32))
